# revision 1
# baseline (speedup 1.0000x reference)
"""Trainium2 Bass kernel for nn_CoAttention_TextDNS.

Math: both additive co-attention blocks have scores of the form
    score[l, m] = f(l) + g(m) + const
followed by softmax over the last axis, so the row-dependent terms cancel
(softmax shift invariance) and the attention weights are identical for every
row l:
    att_dns[b]  = broadcast_rows( softmax(tanh(dns[b]  @ W_d1.T) @ wb) @ dns[b] )
    att_text[b] = broadcast_rows( softmax(tanh(text[b] @ W_t2.T) @ wd) @ text[b] )
with wb = w_att1[H:], wd = w_att2[H:].  W_t1/b_t1/W_d2/b_d2/wa/wc/b_att1/
b_att2 do not affect the output.

Sharding: data-parallel over batch, one batch element per NeuronCore (B=8).
The host pre-transposes every matmul operand so all device DMAs are
contiguous [128, N] block loads; the device computes the two matmuls
(fp32r on the PE), tanh, the wb/wd projections, softmax, and the weighted
row-sums v1/v2; the host broadcasts those rows back to the full
(8, 256, 768) outputs.
"""

import numpy as np

B, L, M, H = 8, 256, 128, 768
HC = H // 128  # 6 contraction chunks of 128


def _build_module(reps=1):
    """Build the per-core module. reps>1 wraps the main pipeline in an
    on-device hardware loop — used only for wall-clock benchmarking (the
    ~70 ms axon dispatch RTT swamps a single ~20 us execution)."""
    import concourse.bass as bass
    import concourse.tile as tile
    from concourse import bacc, mybir
    from concourse.masks import make_identity
    from contextlib import nullcontext

    f32 = mybir.dt.float32
    f32r = mybir.dt.float32r

    nc = bacc.Bacc("TRN2", target_bir_lowering=False, debug=False)

    # Per-core inputs (host-prepared layouts, pre-rounded to the tf32 grid;
    # see kernel()).  PE-consumed tensors are float32r end-to-end so the BIR
    # verifier's fp32r-producer rule is satisfied; the DVE reads them through
    # a bitcast back to plain f32.
    dnst = nc.dram_tensor("dnst", [128, HC * M], f32r, kind="ExternalInput").ap()
    textt = nc.dram_tensor("textt", [128, HC * L], f32r, kind="ExternalInput").ap()
    wd1t = nc.dram_tensor("wd1t", [128, HC * H], f32r, kind="ExternalInput").ap()
    wt2t = nc.dram_tensor("wt2t", [128, HC * H], f32r, kind="ExternalInput").ap()
    wb_in = nc.dram_tensor("wb", [1, H], f32, kind="ExternalInput").ap()
    wd_in = nc.dram_tensor("wd", [1, H], f32, kind="ExternalInput").ap()
    v1_out = nc.dram_tensor("v1", [128, HC], f32, kind="ExternalOutput").ap()
    v2_out = nc.dram_tensor("v2", [128, HC], f32, kind="ExternalOutput").ap()

    Tanh = mybir.ActivationFunctionType.Tanh
    Exp = mybir.ActivationFunctionType.Exp

    with tile.TileContext(nc) as tc:
        with (
            tc.tile_pool(name="ins", bufs=1) as ins,
            tc.tile_pool(name="work", bufs=1) as work,
            tc.tile_pool(name="scratch", bufs=2) as scratch,
            tc.tile_pool(name="mm", bufs=4, space="PSUM") as mm,
            tc.tile_pool(name="prepp", bufs=2, space="PSUM") as prepp,
            tc.tile_pool(name="urowp", bufs=1, space="PSUM") as urowp,
            tc.tile_pool(name="warmp", bufs=1, space="PSUM") as warmp,
        ):
            # ---- constants / small inputs -------------------------------
            ident = ins.tile([128, 128], f32, tag="ident")
            make_identity(nc, ident)
            ones_row = ins.tile([1, 128], f32, tag="ones")
            nc.vector.memset(ones_row, 1.0)

            # Warm the PE (HAM clock gate releases after ~3.4us of sustained
            # activity) with junk identity matmuls while the bulk DMAs
            # stream and the PE would otherwise sit cold.
            warm_ps = warmp.tile([128, 384], f32, tag="warm")
            for _ in range(12):
                nc.tensor.matmul(
                    warm_ps[:, 0:128], ident, ident, start=True, stop=True
                )

            # Tiny wb/wd row loads ride the ACT HWDGE ring; the replicate
            # across 128 partitions runs on the early-idle PE (ones.T @ row)
            # + DVE copies, keeping the DMA engines free for the weight
            # stream (a broadcast DMA costs ~2.2us of shared engine time).
            wb_sb = ins.tile([1, H], f32, tag="wb")
            nc.scalar.dma_start(out=wb_sb, in_=wb_in)
            wd_sb = ins.tile([1, H], f32, tag="wd")
            nc.scalar.dma_start(out=wd_sb, in_=wd_in)
            wb_rep = work.tile([128, H], f32, tag="wb_rep")
            wd_rep = work.tile([128, H], f32, tag="wd_rep")
            for w_sb, w_rep in ((wd_sb, wd_rep), (wb_sb, wb_rep)):
                for half in range(2):
                    sl = slice(half * 384, (half + 1) * 384)
                    rep_ps = warmp.tile([128, 384], f32, tag="warm")
                    nc.tensor.matmul(rep_ps, ones_row, w_sb[:, sl],
                                     start=True, stop=True)
                    nc.vector.tensor_copy(out=w_rep[:, sl], in_=rep_ps)

            # ---- main pipeline (optionally looped for benchmarking) -----
            loop = tc.For_i(0, reps, 1) if reps > 1 else nullcontext()
            with loop:
                _pipeline_body(nc, tc, ins, work, scratch, mm, prepp, urowp,
                               mybir, dnst, textt, wd1t, wt2t, v1_out, v2_out,
                               ident, ones_row, wb_rep, wd_rep,
                               Tanh, Exp, f32, f32r)

    nc.compile()
    return nc


def _pipeline_body(nc, tc, ins, work, scratch, mm, prepp, urowp, mybir,
                   dnst, textt, wd1t, wt2t, v1_out, v2_out,
                   ident, ones_row, wb_rep, wd_rep, Tanh, Exp, f32, f32r):
    # DMA order is the schedule: the t2/v2 chain (256 text rows — the long
    # dependency chain) streams FIRST so its DVE/ACT work overlaps the
    # d1-path weight stream; the short d1 chain is the only tail.
    textt_sb = ins.tile([128, HC, L], f32r, tag="textt")
    textt_r = textt.rearrange("p (c l) -> p c l", c=HC)
    for g in range(2):  # 2 groups of 3 chunks: payload >> per-DMA ring cost
        nc.sync.dma_start(
            out=textt_sb[:, 3 * g : 3 * g + 3, :], in_=textt_r[:, 3 * g : 3 * g + 3, :]
        )
    wt2_sb = ins.tile([128, HC, H], f32r, tag="wt2")
    wt2_r = wt2t.rearrange("p (c o) -> p c o", c=HC)
    for c in range(HC):
        nc.sync.dma_start(out=wt2_sb[:, c, :], in_=wt2_r[:, c, :])
    dnst_sb = ins.tile([128, HC, M], f32r, tag="dnst")
    nc.sync.dma_start(out=dnst_sb, in_=dnst.rearrange("p (c m) -> p c m", c=HC))
    wd1_sb = ins.tile([128, HC, H], f32r, tag="wd1")
    wd1_r = wd1t.rearrange("p (c o) -> p c o", c=HC)
    for c in range(HC):
        nc.sync.dma_start(out=wd1_sb[:, c, :], in_=wd1_r[:, c, :])

    from concourse.tile import add_dep_helper

    def emit_mm_groups(groups, after=None):
        """Emit HC-chunk accumulating matmuls for several PSUM groups with
        the CHUNK loop outermost, so each arriving DMA chunk immediately
        feeds every group (a group-contiguous order would stall the whole
        PE queue on the last chunk per group)."""
        pss = [
            mm.tile([128, 384], f32, tag="mmps", name=f"mmps{i}")
            for i in range(len(groups))
        ]
        last = None
        for c in range(HC):
            for ps, (lhsT_fn, rhs_fn) in zip(pss, groups):
                last = nc.tensor.matmul(
                    ps, lhsT_fn(c), rhs_fn(c),
                    start=(c == 0), stop=(c == HC - 1),
                )
                if after is not None and c == 0:
                    add_dep_helper(
                        last.ins, after.ins, sync=False,
                        reason="keep PE queue in stream order",
                    )
        return pss, last

    def tanh_proj(pss, w_rep, ucol):
        """tanh the two o-half PSUM groups of one 128-row tile, project
        against w_rep, and reduce into ucol [128, 1]."""
        act = scratch.tile([128, H], f32, tag="act")
        prod = scratch.tile([128, H], f32, tag="prod")
        parts = work.tile([128, 2], f32, tag="upart")
        for half, ps in enumerate(pss):
            sl = slice(half * 384, (half + 1) * 384)
            nc.scalar.activation(act[:, sl], ps, Tanh)
            nc.vector.tensor_mul(prod[:, sl], act[:, sl], w_rep[:, sl])
            nc.vector.reduce_sum(
                out=parts[:, half : half + 1], in_=prod[:, sl],
                axis=mybir.AxisListType.X,
            )
        nc.vector.tensor_add(ucol, parts[:, 0:1], parts[:, 1:2])

    def softmax_row(urow, n, prefix):
        """softmax of a [1, n] row (SBUF or PSUM); returns SBUF [1, n].
        No max-subtraction: the additive scores here are O(1), and softmax
        shift-invariance keeps the math identical to the reference."""
        erow = work.tile([1, n], f32, tag=f"{prefix}erow")
        esum = work.tile([1, 1], f32, tag=f"{prefix}esum")
        nc.scalar.activation(out=erow, in_=urow, func=Exp, accum_out=esum)
        rsum = work.tile([1, 1], f32, tag=f"{prefix}rsum")
        nc.vector.reciprocal(out=rsum, in_=esum)
        prow = work.tile([1, n], f32, tag=f"{prefix}prow")
        nc.vector.tensor_scalar_mul(prow, erow, rsum)
        return prow

    def weighted_rowsum(prow, n, xt_sb, v_sb):
        """v[h] = sum_r p[r] * X[r, h] given X.T chunks [128, n].  One mul
        over the whole [128, HC, n] tile (prep broadcast along the chunk dim
        via a step-0 AP) + one X-axis reduce."""
        import concourse.bass as bass

        prep = prepp.tile([128, n], f32, tag="prep")
        prep_mm = nc.tensor.matmul(prep, ones_row, prow, start=True, stop=True)
        hc2 = HC // 2
        prep_b = bass.AP(
            tensor=prep.tensor,
            offset=prep.offset,
            ap=[prep.ap[0], [0, hc2], prep.ap[1]],
        )
        prod = scratch.tile([128, HC, n], f32, tag=f"vprod{n}")
        nc.vector.tensor_mul(prod[:, :hc2, :], xt_sb[:, :hc2, :].bitcast(f32), prep_b)
        nc.vector.tensor_mul(prod[:, hc2:, :], xt_sb[:, hc2:, :].bitcast(f32), prep_b)
        nc.vector.reduce_sum(
            out=v_sb[:, hc2:], in_=prod[:, hc2:, :], axis=mybir.AxisListType.X
        )
        nc.vector.reduce_sum(
            out=v_sb[:, :hc2], in_=prod[:, :hc2, :], axis=mybir.AxisListType.X
        )
        return prep_mm

    # ---- t2 path: scores for the 256 text rows (two 128-row tiles) ----
    u2col = work.tile([128, 2], f32, tag="u2col")
    t2_groups = [
        (
            lambda c, lt=lt: textt_sb[:, c, lt * 128 : (lt + 1) * 128],
            lambda c, half=half: wt2_sb[:, c, half * 384 : (half + 1) * 384],
        )
        for lt in range(2)
        for half in range(2)
    ]
    t2_pss, t2_last_mm = emit_mm_groups(t2_groups)
    for lt in range(2):
        tanh_proj(t2_pss[2 * lt : 2 * lt + 2], wd_rep, u2col[:, lt : lt + 1])
    u2sb = work.tile([1, 256], f32, tag="u2sb")
    for lt in range(2):
        u2row = urowp.tile([1, 128], f32, tag="urow")
        nc.tensor.transpose(u2row, u2col[:, lt : lt + 1], ident)
        nc.vector.tensor_copy(out=u2sb[:, lt * 128 : (lt + 1) * 128], in_=u2row)
    p2row = softmax_row(u2sb, 256, "p2")

    # ---- d1 path: scores for the 128 dns rows (the true tail chain; its
    # DVE ops are emitted before v2's so they win the FIFO) --------------
    u1col = work.tile([128, 1], f32, tag="u1col")
    d1_groups = [
        (
            lambda c: dnst_sb[:, c, :],
            lambda c, half=half: wd1_sb[:, c, half * 384 : (half + 1) * 384],
        )
        for half in range(2)
    ]
    d1_pss, _ = emit_mm_groups(d1_groups)
    tanh_proj(d1_pss, wb_rep, u1col)
    u1row = urowp.tile([1, 128], f32, tag="urow")
    nc.tensor.transpose(u1row, u1col, ident)
    p1row = softmax_row(u1row, 128, "p1")

    # ---- weighted row sums (fill DVE after the score chains) ----------
    v2_sb = work.tile([128, HC], f32, tag="v2sb")
    weighted_rowsum(p2row, 256, textt_sb, v2_sb)
    nc.sync.dma_start(out=v2_out, in_=v2_sb)
    v1_sb = work.tile([128, HC], f32, tag="v1sb")
    weighted_rowsum(p1row, 128, dnst_sb, v1_sb)
    nc.sync.dma_start(out=v1_out, in_=v1_sb)


_NC_CACHE = {}


def _get_module(reps=1):
    if reps not in _NC_CACHE:
        _NC_CACHE[reps] = _build_module(reps)
    return _NC_CACHE[reps]


def _round_tf32(x):
    """Round fp32 to the tf32 grid (10 mantissa bits, round-to-nearest-even)
    so the PE's fp32r read sees already-representable values."""
    u = np.ascontiguousarray(x, np.float32).view(np.uint32)
    r = (u + np.uint32(0x0FFF) + ((u >> np.uint32(13)) & np.uint32(1))) & np.uint32(
        0xFFFFE000
    )
    return r.view(np.float32)


def _chunked_T(x, inner):
    """[R, H] -> [128, HC*inner] with [p, c*inner + r] = x[r, c*128 + p]."""
    r = x.shape[0]
    assert x.shape == (r, H) and r == inner
    return _round_tf32(
        x.T.reshape(HC, 128, inner).transpose(1, 0, 2).reshape(128, HC * inner)
    )


def _make_in_maps(kernel_inputs):
    text = np.asarray(kernel_inputs["text_features"], np.float32)
    dns = np.asarray(kernel_inputs["dns_features"], np.float32)
    W_d1 = np.asarray(kernel_inputs["W_d1"], np.float32)
    W_t2 = np.asarray(kernel_inputs["W_t2"], np.float32)
    wb = np.ascontiguousarray(
        np.asarray(kernel_inputs["w_att1"], np.float32)[H:].reshape(1, H)
    )
    wd = np.ascontiguousarray(
        np.asarray(kernel_inputs["w_att2"], np.float32)[H:].reshape(1, H)
    )
    wd1t = _chunked_T(W_d1, H)  # [p, c*H + o] = W_d1[o, c*128 + p]
    wt2t = _chunked_T(W_t2, H)

    in_maps = []
    for b in range(B):
        in_maps.append(
            {
                "dnst": _chunked_T(dns[b], M),
                "textt": _chunked_T(text[b], L),
                "wd1t": wd1t,
                "wt2t": wt2t,
                "wb": wb,
                "wd": wd,
            }
        )
    return in_maps


def _run_device(kernel_inputs):
    from concourse.bass_utils import run_bass_kernel_spmd

    in_maps = _make_in_maps(kernel_inputs)
    nc = _get_module()
    return run_bass_kernel_spmd(nc, in_maps, list(range(B)))


def kernel(**inputs):
    res = _run_device(inputs)
    att_text = np.empty((B, L, H), np.float32)
    att_dns = np.empty((B, L, H), np.float32)
    for b in range(B):
        r = res.results[b]
        v1 = r["v1"].T.reshape(H)  # [128, HC] -> [H]
        v2 = r["v2"].T.reshape(H)
        att_dns[b] = v1[None, :]
        att_text[b] = v2[None, :]
    return att_text, att_dns

